# revision 31
# baseline (speedup 1.0000x reference)
"""MixerBlock kernel for 8 Trainium2 NeuronCores.

Problem (hardcoded shapes): x (4, 1024, 2048) f32; per-head causal mixing.

  xt = x^T @ in_w.T + in_b                      # (B, S, E)
  p  = heads(xt)                                # (B, H, e, S), c = h*64+e
  col heads h<8:  out[c,s] = v_h[s] * sum_{t<=s} p[c,t]
  row heads h>=8: out[c,s] = sum_{t<=s} v_h[t] * p[c,t]
  out = merge @ out_w.T + out_b, transposed back to (B, E, S)

The S x S mixing matrices are rank-structured causal, so the einsum collapses
to a cumulative sum along t with a per-head pre-scale (row heads) or
post-scale (col heads).

Sharding: 8 cores = (batch b in 0..3) x (channel-group g in {0,1}).  Each
group holds 4 col heads AND 4 row heads (g=0: heads 0-3 + 8-11; g=1: heads
4-7 + 12-15), so within one core m-tiles 0,1 are col-type and m-tiles 2,3
are row-type.  This removes the multiply-by-ones passes a col-only/row-only
split would need: col tiles cumsum straight out of PSUM then post-scale;
row tiles pre-scale out of PSUM then cumsum straight into the bf16 mixed
tile.  Each core computes in_proj for its 512 channels, the causal mixing
(vector-engine tensor_tensor_scan along the free dim), and a partial
out_proj over its channel slice, producing a full-size (E, S) bf16 partial.
Host sums the two partials per batch in f32.  No cross-core communication.

All matmul operands are bf16 (inputs cast on host): the PE runs bf16 at the
same 1 cycle/row as f32r but fast-weight-load works, and every DMA stream
(x, weights, pre/post broadcast expands, output) halves.  PSUM accumulation
and the scan state stay f32; the rel-err budget (2e-2) dwarfs the bf16
quantization (~5e-3 observed).

Biases (all zero in setup_inputs) enter linearly and are folded in on the
host via a closed form when nonzero.
"""

import numpy as np
from ml_dtypes import bfloat16

B, E, S, H = 4, 1024, 2048, 16
C = 512          # channels per core (8 heads x 64)
P = 128
NK = E // P      # 8 contraction tiles for in_proj
NM = C // P      # 4 local-channel tiles (0,1 col-type; 2,3 row-type)
NHALF = 2        # t halves for x streaming
TQ = S // NHALF  # 1024
NQ = TQ // 512   # 2 512-chunks per half
NEO = E // P     # 8 output-row tiles
NS = S // 512    # 4 512-wide s slices
N_CORES = 8

_NC = None


def _build_nc():
    from contextlib import ExitStack

    import concourse.bacc as bacc
    import concourse.mybir as mybir
    import concourse.tile as tile
    from concourse.alu_op_type import AluOpType
    from concourse.tile import add_dep_helper

    f32 = mybir.dt.float32
    bf16 = mybir.dt.bfloat16

    nc = bacc.Bacc(
        "TRN2",
        target_bir_lowering=False,
        debug=False,
        enable_asserts=True,
        num_devices=N_CORES,
    )
    x_d = nc.dram_tensor("x", (E, S), bf16, kind="ExternalInput").ap()
    win_d = nc.dram_tensor("w_in", (E, C), bf16, kind="ExternalInput").ap()
    wout_d = nc.dram_tensor("w_out", (C, E), bf16, kind="ExternalInput").ap()
    pre_d = nc.dram_tensor("pre", (4, S), bf16, kind="ExternalInput").ap()
    post_d = nc.dram_tensor("post", (4, S), bf16, kind="ExternalInput").ap()
    out_d = nc.dram_tensor("out", (E, S), bf16, kind="ExternalOutput").ap()

    xt = x_d.rearrange("(ko p) t -> p ko t", p=P)        # (128, 8, 2048)
    wi = win_d.rearrange("(ko p) c -> p ko c", p=P)      # (128, 8, 512)
    wo = wout_d.rearrange("(kc p) eo -> p kc eo", p=P)   # (128, 4, 1024)
    outr = out_d.rearrange("(mo p) s -> p mo s", p=P)    # (128, 8, 2048)

    with tile.TileContext(nc) as tc:
        with ExitStack() as ctx:
            wpool = ctx.enter_context(tc.tile_pool(name="w", bufs=1))
            xpool = ctx.enter_context(tc.tile_pool(name="xc", bufs=1))
            scpool = ctx.enter_context(tc.tile_pool(name="sc", bufs=1))
            qpool = ctx.enter_context(tc.tile_pool(name="q", bufs=4))
            cumpool = ctx.enter_context(tc.tile_pool(name="cum", bufs=1))
            mixpool = ctx.enter_context(tc.tile_pool(name="mix", bufs=1))
            outpool = ctx.enter_context(tc.tile_pool(name="o", bufs=4))
            pp = ctx.enter_context(tc.tile_pool(name="pp", bufs=8, space="PSUM"))

            w_in_sb = wpool.tile([P, NK, C], bf16, tag="w_in")

            # x streamed in s-quarters of 512 (matching the psum quarter
            # pipeline below): each quarter is 0.5 MB against ~6.8us of PE
            # work, so the stream can never starve the PE.  xsem holds the
            # w_out transfer back until the x stream is mostly in.
            xsem = nc.alloc_semaphore("xsem")
            xs = xpool.tile([P, NK, S], bf16, tag="xs", name="xs")
            # x lands in (ko-pair, t-half) chunks — 2 KB contiguous lines,
            # the efficient DMA granularity — while the compute below reads
            # 512-wide s-quarter subtiles of them.  Fine-grained leading
            # chunks of w_in and x so quarter 0's first matmul is gated on
            # ~0.4 MB; the first w chunks ride the (idle) scalar queue so
            # they land concurrently with the first x chunk.
            nc.scalar.dma_start(w_in_sb[:, 0:1, :], wi[:, 0:1, :])
            nc.sync.dma_start(xs[:, 0:1, 0:TQ], xt[:, 0:1, 0:TQ])
            nc.sync.dma_start(xs[:, 1:2, 0:TQ], xt[:, 1:2, 0:TQ])
            nc.sync.dma_start(w_in_sb[:, 1:2, :], wi[:, 1:2, :])
            nc.sync.dma_start(w_in_sb[:, 2:4, :], wi[:, 2:4, :])
            nc.sync.dma_start(xs[:, 2:4, 0:TQ], xt[:, 2:4, 0:TQ])
            nc.sync.dma_start(w_in_sb[:, 4:8, :], wi[:, 4:8, :])
            for kp in range(2, NK // 2):
                nc.sync.dma_start(xs[:, 2 * kp:2 * kp + 2, 0:TQ],
                                  xt[:, 2 * kp:2 * kp + 2, 0:TQ])
            for kp in range(NK // 2):
                nc.sync.dma_start(xs[:, 2 * kp:2 * kp + 2, TQ:S],
                                  xt[:, 2 * kp:2 * kp + 2, TQ:S])

            # w_out is not needed until out_proj; it rides the scalar
            # HWDGE queue, gated until in_proj is half done so it never
            # competes with the x stream.
            w_out_sb = wpool.tile([P, NM, E], bf16, tag="w_out")
            nc.scalar.dma_start(w_out_sb[:], wo)._wait_ge(xsem, 1)

            # scan's data1 is ignored under op1=bypass but must be a valid
            # (allocated) SBUF operand of matching shape/dtype.  Both
            # memsets go FIRST on the gpsimd queue (they are ~0.1us each
            # and the warm-up matmuls below gate on them); the expand
            # DMA-gens queue behind.
            dummy = scpool.tile([P, 512], f32, tag="dummy")
            nc.gpsimd.memset(dummy[:], 0.0)
            # The PE's HAM clock gate holds it at 1.2 GHz until it has
            # been busy ~3.4us.  The first real matmul can't start before
            # the x/w head chunks land (~11us), so burn the wait warming
            # the array on junk data; the real matmuls then run at
            # 2.4 GHz from the first one.
            warm = scpool.tile([P, 512], bf16, tag="warm")
            nc.gpsimd.memset(warm[:], 0.0)
            wps = pp.tile([P, 512], f32, tag="ps", name="warmps")
            for _ in range(8):
                nc.tensor.matmul(wps[:], lhsT=warm[:, 0:P], rhs=warm[:],
                                 start=True, stop=True)

            # pre (row tiles m=2,3) / post (col tiles m=0,1) expanded to
            # 128 partitions: each 64-channel head gets its mix row
            # replicated across its 64 partitions.  Ungated: the SWDGE
            # (gpsimd) queue starts these at kernel start — they are tiny
            # (1 MB total) and needed by the first quarter's DVE ops.
            def expand_heads(rows_ap, tag):
                t = scpool.tile([P, S], bf16, tag=tag, name=tag)
                for j in range(2):
                    nc.gpsimd.dma_start(
                        t[j * 64:(j + 1) * 64, :],
                        rows_ap[j].partition_broadcast(64),
                    )
                return t

            post_t = {m: expand_heads(post_d[2 * m:2 * m + 2, :], f"post{m}")
                      for m in range(2)}
            pre_t = {m: expand_heads(pre_d[2 * (m - 2):2 * (m - 2) + 2, :],
                                     f"pre{m}")
                     for m in range(2, NM)}

            cum = {m: cumpool.tile([P, S], f32, tag=f"cum{m}", name=f"cum{m}")
                   for m in range(2)}
            mixed = [mixpool.tile([P, S], bf16, tag=f"mx{m}", name=f"mx{m}")
                     for m in range(NM)]
            # ---- in_proj (bf16) + per-type mixing, streamed in t halves ----
            for h in range(NHALF):
                lo = h * TQ
                ps = {(m, n): pp.tile([P, 512], f32, tag="ps",
                                      name=f"pp{h}_{m}_{n}")
                      for m in range(NM) for n in range(NQ)}
                # half 0: k-outer for ko 0-5 — the PE consumes x chunks in
                # DMA arrival order, so it can't starve mid-half — then
                # m-outer over the last two ko, handing each m's completed
                # psums to the DVE ~2.5us before the half would otherwise
                # end.  half 1: m-outer — x h1 is fully resident by then,
                # and m-order psum completion both matches the DVE chain
                # order and recycles each m's banks just after the DVE h0
                # chain frees them.
                if h == 0:
                    loop = [(ko, m, n) for ko in range(NK - 2)
                            for m in range(NM) for n in range(NQ)] + \
                           [(ko, m, n) for m in range(NM)
                            for ko in range(NK - 2, NK) for n in range(NQ)]
                else:
                    loop = [(ko, m, n) for m in range(NM)
                            for ko in range(NK) for n in range(NQ)]
                for ko, m, n in loop:
                    mm_ret = nc.tensor.matmul(
                        ps[(m, n)][:],
                        lhsT=w_in_sb[:, ko, m * P:(m + 1) * P],
                        rhs=xs[:, ko, lo + n * 512:lo + (n + 1) * 512],
                        start=(ko == 0),
                        stop=(ko == NK - 1),
                    )
                    if h == 1 and ko == 0 and m == 0 and n == 0:
                        rel = nc.tensor.sem_inc(xsem, 1)
                        add_dep_helper(rel.ins, mm_ret.ins, sync=False,
                                       reason="xsem gate: release w_out")

                # DVE ops in m-tile completion order, each m's chain run to
                # its end before the next starts: psums free for recycling
                # ASAP and every mixed[m] half completes at the earliest
                # point the data allows.
                for m in range(2):          # col: cumsum straight from PSUM
                    for n in range(NQ):
                        at = lo + n * 512
                        init = 0.0 if (h == 0 and n == 0) \
                            else cum[m][:, at - 1:at]
                        nc.vector.tensor_tensor_scan(
                            cum[m][:, at:at + 512], ps[(m, n)][:], dummy[:],
                            init, AluOpType.add, AluOpType.bypass,
                        )
                    # post-scale -> bf16 mixed; SBUF-only operands, so it
                    # runs on the (otherwise idle) GPSIMD engine, off the
                    # DVE critical path
                    nc.gpsimd.tensor_tensor(
                        mixed[m][:, lo:lo + TQ], cum[m][:, lo:lo + TQ],
                        post_t[m][:, lo:lo + TQ], AluOpType.mult,
                    )
                for m in range(2, NM):      # row: pre-scale from PSUM
                    q = qpool.tile([P, TQ], f32, tag="q", name=f"q{m}_{h}")
                    for n in range(NQ):
                        nc.vector.tensor_tensor(
                            q[:, n * 512:(n + 1) * 512],
                            ps[(m, n)][:],
                            pre_t[m][:, lo + n * 512:lo + (n + 1) * 512],
                            AluOpType.mult,
                        )
                    # cumsum -> bf16 mixed
                    init = 0.0 if h == 0 else mixed[m][:, TQ - 1:TQ]
                    nc.vector.tensor_tensor_scan(
                        mixed[m][:, lo:lo + TQ], q[:], q[:],
                        init, AluOpType.add, AluOpType.bypass,
                    )

            # ---- out_proj (partial over this core's 512 channels) ----
            # s-outer: s slices 0/1 (first-half mixed, ready early) run
            # while the DVE mixing chain still produces the second half,
            # so the PE never stalls on the late mixed tiles
            for so in range(NS):
                for mo in range(NEO):
                    pt = pp.tile([P, 512], f32, tag="ps", name=f"po{so}_{mo}")
                    for kc in range(NM):
                        nc.tensor.matmul(
                            pt[:],
                            lhsT=w_out_sb[:, kc, mo * P:(mo + 1) * P],
                            rhs=mixed[kc][:, so * 512:(so + 1) * 512],
                            start=(kc == 0),
                            stop=(kc == NM - 1),
                        )
                    ot = outpool.tile([P, 512], bf16, tag="o",
                                      name=f"o{so}_{mo}")
                    # ACT casts while the DVE still runs the mixing chain
                    # (casts queued on its strict FIFO would block the h1
                    # chain out_proj s2/s3 wait on); split ACT/DVE once the
                    # mixing chain is drained
                    if so < 2 or mo % 2 == 0:
                        nc.scalar.copy(out=ot[:], in_=pt[:])
                    else:
                        nc.vector.tensor_copy(out=ot[:], in_=pt[:])
                    # alternate the two HWDGE rings for the output stream
                    eng = nc.sync if mo % 2 == 0 else nc.scalar
                    eng.dma_start(
                        outr[:, mo, so * 512:(so + 1) * 512], ot[:])
    nc.compile()
    return nc


def _get_nc():
    global _NC
    if _NC is None:
        _NC = _build_nc()
    return _NC


def _core_channels(g):
    """Full-E channel indices owned by group g: 4 col heads + 4 row heads."""
    return np.r_[np.arange(g * 256, (g + 1) * 256),
                 np.arange(512 + g * 256, 512 + (g + 1) * 256)]


def shard_inputs(x, in_w, out_w, mix_w):
    x = np.ascontiguousarray(x, np.float32)
    in_w = np.asarray(in_w, np.float32)
    out_w = np.asarray(out_w, np.float32)
    mix_w = np.ascontiguousarray(mix_w, np.float32)
    group = []
    for g in range(2):
        ch = _core_channels(g)
        group.append({
            "w_in": np.ascontiguousarray(in_w[ch, :].T).astype(bfloat16),
            "w_out": np.ascontiguousarray(out_w[:, ch].T).astype(bfloat16),
            "pre": mix_w[8 + 4 * g:12 + 4 * g].astype(bfloat16),
            "post": mix_w[4 * g:4 + 4 * g].astype(bfloat16),
        })
    xb = [np.ascontiguousarray(x[b]).astype(bfloat16) for b in range(B)]
    in_maps = []
    for b in range(B):
        for g in range(2):
            m = {"x": xb[b]}
            m.update(group[g])
            in_maps.append(m)
    return in_maps


def _bias_contribution(in_b, out_b, mix_b, mix_w, out_w):
    """Closed-form (E, S) addend from the (linear) bias terms."""
    if not (np.any(in_b) or np.any(out_b) or np.any(mix_b)):
        return None
    s_idx = np.arange(S, dtype=np.float64)
    bias1 = np.zeros((E, S), np.float64)
    for h in range(H):
        cs = slice(h * 64, (h + 1) * 64)
        v = np.asarray(mix_w[h], np.float64)
        if h < H // 2:
            g = (s_idx + 1.0) * v          # cumsum of constant, then *v[s]
        else:
            g = np.cumsum(v)               # cumsum of v[t]
        bias1[cs] = np.asarray(in_b, np.float64)[cs, None] * g[None, :]
        bias1[cs] += np.asarray(mix_b[h], np.float64)[None, :]
    fb = np.asarray(out_w, np.float64) @ bias1
    fb += np.asarray(out_b, np.float64)[:, None]
    return fb.astype(np.float32)


def run_sharded(in_maps, trace=False):
    from concourse.bass_utils import run_bass_kernel_spmd

    return run_bass_kernel_spmd(
        _get_nc(), in_maps, core_ids=list(range(N_CORES)), trace=trace
    )


def gather_output(results, bias_fb=None):
    out = np.empty((B, E, S), np.float32)
    for b in range(B):
        out[b] = results[2 * b]["out"].astype(np.float32)
        out[b] += results[2 * b + 1]["out"].astype(np.float32)
        if bias_fb is not None:
            out[b] += bias_fb
    return out


def kernel(x, in_w, in_b, out_w, out_b, mix_w, mix_b):
    in_maps = shard_inputs(x, in_w, out_w, mix_w)
    res = run_sharded(in_maps, trace=False)
    fb = _bias_contribution(
        np.asarray(in_b), np.asarray(out_b), np.asarray(mix_b),
        np.asarray(mix_w), np.asarray(out_w))
    return gather_output(res.results, fb)


# revision 34
# speedup vs baseline: 1.0991x; 1.0991x over previous
"""MixerBlock kernel for 8 Trainium2 NeuronCores.

Problem (hardcoded shapes): x (4, 1024, 2048) f32; per-head causal mixing.

  xt = x^T @ in_w.T + in_b                      # (B, S, E)
  p  = heads(xt)                                # (B, H, e, S), c = h*64+e
  col heads h<8:  out[c,s] = v_h[s] * sum_{t<=s} p[c,t]
  row heads h>=8: out[c,s] = sum_{t<=s} v_h[t] * p[c,t]
  out = merge @ out_w.T + out_b, transposed back to (B, E, S)

The S x S mixing matrices are rank-structured causal, so the einsum collapses
to a cumulative sum along t with a per-head pre-scale (row heads) or
post-scale (col heads).

Sharding: 8 cores = (batch b in 0..3) x (channel-group g in {0,1}).  Each
group holds 4 col heads AND 4 row heads (g=0: heads 0-3 + 8-11; g=1: heads
4-7 + 12-15), so within one core m-tiles 0,1 are col-type and m-tiles 2,3
are row-type.  This removes the multiply-by-ones passes a col-only/row-only
split would need: col tiles cumsum straight out of PSUM then post-scale;
row tiles pre-scale out of PSUM then cumsum straight into the bf16 mixed
tile.  Each core computes in_proj for its 512 channels, the causal mixing
(vector-engine tensor_tensor_scan along the free dim), and a partial
out_proj over its channel slice, producing a full-size (E, S) bf16 partial.
Host sums the two partials per batch in f32.  No cross-core communication.

All matmul operands are bf16 (inputs cast on host): the PE runs bf16 at the
same 1 cycle/row as f32r but fast-weight-load works, and every DMA stream
(x, weights, pre/post broadcast expands, output) halves.  PSUM accumulation
and the scan state stay f32; the rel-err budget (2e-2) dwarfs the bf16
quantization (~5e-3 observed).

Biases (all zero in setup_inputs) enter linearly and are folded in on the
host via a closed form when nonzero.
"""

import numpy as np
from ml_dtypes import bfloat16

B, E, S, H = 4, 1024, 2048, 16
C = 512          # channels per core (8 heads x 64)
P = 128
NK = E // P      # 8 contraction tiles for in_proj
NM = C // P      # 4 local-channel tiles (0,1 col-type; 2,3 row-type)
NHALF = 2        # t halves for x streaming
TQ = S // NHALF  # 1024
NQ = TQ // 512   # 2 512-chunks per half
NEO = E // P     # 8 output-row tiles
NS = S // 512    # 4 512-wide s slices
N_CORES = 8

_NC = None


def _build_nc():
    from contextlib import ExitStack

    import concourse.bacc as bacc
    import concourse.mybir as mybir
    import concourse.tile as tile
    from concourse.alu_op_type import AluOpType
    from concourse.tile import add_dep_helper

    f32 = mybir.dt.float32
    bf16 = mybir.dt.bfloat16

    nc = bacc.Bacc(
        "TRN2",
        target_bir_lowering=False,
        debug=False,
        enable_asserts=True,
        num_devices=N_CORES,
    )
    x_d = nc.dram_tensor("x", (E, S), bf16, kind="ExternalInput").ap()
    win_d = nc.dram_tensor("w_in", (E, C), bf16, kind="ExternalInput").ap()
    wout_d = nc.dram_tensor("w_out", (C, E), bf16, kind="ExternalInput").ap()
    pre_d = nc.dram_tensor("pre", (4, S), bf16, kind="ExternalInput").ap()
    post_d = nc.dram_tensor("post", (4, S), bf16, kind="ExternalInput").ap()
    out_d = nc.dram_tensor("out", (E, S), bf16, kind="ExternalOutput").ap()

    xt = x_d.rearrange("(ko p) t -> p ko t", p=P)        # (128, 8, 2048)
    wi = win_d.rearrange("(ko p) c -> p ko c", p=P)      # (128, 8, 512)
    wo = wout_d.rearrange("(kc p) eo -> p kc eo", p=P)   # (128, 4, 1024)
    outr = out_d.rearrange("(mo p) s -> p mo s", p=P)    # (128, 8, 2048)

    with tile.TileContext(nc) as tc:
        with ExitStack() as ctx:
            wpool = ctx.enter_context(tc.tile_pool(name="w", bufs=1))
            xpool = ctx.enter_context(tc.tile_pool(name="xc", bufs=1))
            scpool = ctx.enter_context(tc.tile_pool(name="sc", bufs=1))
            qpool = ctx.enter_context(tc.tile_pool(name="q", bufs=4))
            cumpool = ctx.enter_context(tc.tile_pool(name="cum", bufs=1))
            mixpool = ctx.enter_context(tc.tile_pool(name="mix", bufs=1))
            outpool = ctx.enter_context(tc.tile_pool(name="o", bufs=4))
            pp = ctx.enter_context(tc.tile_pool(name="pp", bufs=8, space="PSUM"))

            w_in_sb = wpool.tile([P, NK, C], bf16, tag="w_in")

            # x streamed in s-quarters of 512 (matching the psum quarter
            # pipeline below): each quarter is 0.5 MB against ~6.8us of PE
            # work, so the stream can never starve the PE.  xsem holds the
            # w_out transfer back until the x stream is mostly in.
            xsem = nc.alloc_semaphore("xsem")
            xs = xpool.tile([P, NK, S], bf16, tag="xs", name="xs")
            # x lands in (ko-pair, t-half) chunks — 2 KB contiguous lines,
            # the efficient DMA granularity — while the compute below reads
            # 512-wide s-quarter subtiles of them.  Fine-grained leading
            # chunks of w_in and x so quarter 0's first matmul is gated on
            # ~0.4 MB; the first w chunks ride the (idle) scalar queue so
            # they land concurrently with the first x chunk.
            nc.scalar.dma_start(w_in_sb[:, 0:1, :], wi[:, 0:1, :])
            nc.sync.dma_start(xs[:, 0:1, 0:TQ], xt[:, 0:1, 0:TQ])
            nc.sync.dma_start(xs[:, 1:2, 0:TQ], xt[:, 1:2, 0:TQ])
            nc.sync.dma_start(w_in_sb[:, 1:2, :], wi[:, 1:2, :])
            nc.sync.dma_start(w_in_sb[:, 2:4, :], wi[:, 2:4, :])
            nc.sync.dma_start(xs[:, 2:4, 0:TQ], xt[:, 2:4, 0:TQ])
            nc.sync.dma_start(w_in_sb[:, 4:8, :], wi[:, 4:8, :])
            for kp in range(2, NK // 2):
                nc.sync.dma_start(xs[:, 2 * kp:2 * kp + 2, 0:TQ],
                                  xt[:, 2 * kp:2 * kp + 2, 0:TQ])
            for kp in range(NK // 2):
                nc.sync.dma_start(xs[:, 2 * kp:2 * kp + 2, TQ:S],
                                  xt[:, 2 * kp:2 * kp + 2, TQ:S])

            # w_out is not needed until out_proj; it rides the scalar
            # HWDGE queue, gated until in_proj's second half so it never
            # competes with the x stream.
            w_out_sb = wpool.tile([P, NM, E], bf16, tag="w_out")
            nc.scalar.dma_start(w_out_sb[:], wo)._wait_ge(xsem, 2)

            # scan's data1 is ignored under op1=bypass but must be a valid
            # (allocated) SBUF operand of matching shape/dtype.  Both
            # memsets go FIRST on the gpsimd queue (they are ~0.1us each
            # and the warm-up matmuls below gate on them); the expand
            # DMA-gens queue behind.
            dummy = scpool.tile([P, 512], f32, tag="dummy")
            nc.gpsimd.memset(dummy[:], 0.0)
            # The PE's HAM clock gate holds it at 1.2 GHz until it has
            # been busy ~3.4us.  The first real matmul can't start before
            # the x/w head chunks land (~11us), so burn the wait warming
            # the array on junk data; the real matmuls then run at
            # 2.4 GHz from the first one.
            warm = scpool.tile([P, 512], bf16, tag="warm")
            nc.gpsimd.memset(warm[:], 0.0)
            wps = pp.tile([P, 512], f32, tag="ps", name="warmps")
            for _ in range(8):
                nc.tensor.matmul(wps[:], lhsT=warm[:, 0:P], rhs=warm[:],
                                 start=True, stop=True)

            # pre (row tiles m=2,3) / post (col tiles m=0,1) expanded to
            # 128 partitions: each 64-channel head gets its mix row
            # replicated across its 64 partitions.  Gated on xsem>=1
            # (mid-h0): starting them at kernel entry steals SDMA
            # bandwidth from the critical x/w_in stream and the whole
            # timeline slips.
            def expand_heads(rows_ap, tag):
                t = scpool.tile([P, S], bf16, tag=tag, name=tag)
                for j in range(2):
                    nc.gpsimd.dma_start(
                        t[j * 64:(j + 1) * 64, :],
                        rows_ap[j].partition_broadcast(64),
                    )._wait_ge(xsem, 1)
                return t

            post_t = {m: expand_heads(post_d[2 * m:2 * m + 2, :], f"post{m}")
                      for m in range(2)}
            pre_t = {m: expand_heads(pre_d[2 * (m - 2):2 * (m - 2) + 2, :],
                                     f"pre{m}")
                     for m in range(2, NM)}

            cum = {m: cumpool.tile([P, S], f32, tag=f"cum{m}", name=f"cum{m}")
                   for m in range(2)}
            mixed = [mixpool.tile([P, S], bf16, tag=f"mx{m}", name=f"mx{m}")
                     for m in range(NM)]
            # ---- in_proj (bf16) + per-type mixing, streamed in t halves ----
            for h in range(NHALF):
                lo = h * TQ
                ps = {(m, n): pp.tile([P, 512], f32, tag="ps",
                                      name=f"pp{h}_{m}_{n}")
                      for m in range(NM) for n in range(NQ)}
                # half 0: k-outer for ko 0-5 — the PE consumes x chunks in
                # DMA arrival order, so it can't starve mid-half — then
                # m-outer over the last two ko, handing each m's completed
                # psums to the DVE ~2.5us before the half would otherwise
                # end.  half 1: m-outer — x h1 is fully resident by then,
                # and m-order psum completion both matches the DVE chain
                # order and recycles each m's banks just after the DVE h0
                # chain frees them.
                if h == 0:
                    loop = [(ko, m, n) for ko in range(NK - 2)
                            for m in range(NM) for n in range(NQ)] + \
                           [(ko, m, n) for m in range(NM)
                            for ko in range(NK - 2, NK) for n in range(NQ)]
                else:
                    loop = [(ko, m, n) for m in range(NM)
                            for ko in range(NK) for n in range(NQ)]
                for ko, m, n in loop:
                    mm_ret = nc.tensor.matmul(
                        ps[(m, n)][:],
                        lhsT=w_in_sb[:, ko, m * P:(m + 1) * P],
                        rhs=xs[:, ko, lo + n * 512:lo + (n + 1) * 512],
                        start=(ko == 0),
                        stop=(ko == NK - 1),
                    )
                    # xsem releases: expands mid-h0 (x h0 consumed, only
                    # the xh1 tail still streaming), w_out at h1 start
                    if h == 0 and ko == NK - 4 and m == 0 and n == 0:
                        rel = nc.tensor.sem_inc(xsem, 1)
                        add_dep_helper(rel.ins, mm_ret.ins, sync=False,
                                       reason="xsem gate: release expands")
                    if h == 1 and ko == 0 and m == 0 and n == 0:
                        rel = nc.tensor.sem_inc(xsem, 1)
                        add_dep_helper(rel.ins, mm_ret.ins, sync=False,
                                       reason="xsem gate: release w_out")

                # DVE ops in m-tile completion order, each m's chain run to
                # its end before the next starts: psums free for recycling
                # ASAP and every mixed[m] half completes at the earliest
                # point the data allows.
                for m in range(2):          # col: cumsum straight from PSUM
                    for n in range(NQ):
                        at = lo + n * 512
                        init = 0.0 if (h == 0 and n == 0) \
                            else cum[m][:, at - 1:at]
                        nc.vector.tensor_tensor_scan(
                            cum[m][:, at:at + 512], ps[(m, n)][:], dummy[:],
                            init, AluOpType.add, AluOpType.bypass,
                        )
                    # post-scale -> bf16 mixed; SBUF-only operands, so it
                    # runs on the (otherwise idle) GPSIMD engine, off the
                    # DVE critical path
                    nc.gpsimd.tensor_tensor(
                        mixed[m][:, lo:lo + TQ], cum[m][:, lo:lo + TQ],
                        post_t[m][:, lo:lo + TQ], AluOpType.mult,
                    )
                for m in range(2, NM):      # row: pre-scale from PSUM
                    q = qpool.tile([P, TQ], f32, tag="q", name=f"q{m}_{h}")
                    for n in range(NQ):
                        nc.vector.tensor_tensor(
                            q[:, n * 512:(n + 1) * 512],
                            ps[(m, n)][:],
                            pre_t[m][:, lo + n * 512:lo + (n + 1) * 512],
                            AluOpType.mult,
                        )
                    # cumsum -> bf16 mixed
                    init = 0.0 if h == 0 else mixed[m][:, TQ - 1:TQ]
                    nc.vector.tensor_tensor_scan(
                        mixed[m][:, lo:lo + TQ], q[:], q[:],
                        init, AluOpType.add, AluOpType.bypass,
                    )

            # ---- out_proj (partial over this core's 512 channels) ----
            # s-outer: s slices 0/1 (first-half mixed, ready early) run
            # while the DVE mixing chain still produces the second half,
            # so the PE never stalls on the late mixed tiles
            for so in range(NS):
                for mo in range(NEO):
                    pt = pp.tile([P, 512], f32, tag="ps", name=f"po{so}_{mo}")
                    for kc in range(NM):
                        nc.tensor.matmul(
                            pt[:],
                            lhsT=w_out_sb[:, kc, mo * P:(mo + 1) * P],
                            rhs=mixed[kc][:, so * 512:(so + 1) * 512],
                            start=(kc == 0),
                            stop=(kc == NM - 1),
                        )
                    ot = outpool.tile([P, 512], bf16, tag="o",
                                      name=f"o{so}_{mo}")
                    # ACT casts while the DVE still runs the mixing chain
                    # (casts queued on its strict FIFO would block the h1
                    # chain out_proj s2/s3 wait on); split ACT/DVE once the
                    # mixing chain is drained
                    if so < 2 or mo % 2 == 0:
                        nc.scalar.copy(out=ot[:], in_=pt[:])
                    else:
                        nc.vector.tensor_copy(out=ot[:], in_=pt[:])
                    # alternate the two HWDGE rings for the output stream
                    eng = nc.sync if mo % 2 == 0 else nc.scalar
                    eng.dma_start(
                        outr[:, mo, so * 512:(so + 1) * 512], ot[:])
    nc.compile()
    return nc


def _get_nc():
    global _NC
    if _NC is None:
        _NC = _build_nc()
    return _NC


def _core_channels(g):
    """Full-E channel indices owned by group g: 4 col heads + 4 row heads."""
    return np.r_[np.arange(g * 256, (g + 1) * 256),
                 np.arange(512 + g * 256, 512 + (g + 1) * 256)]


def shard_inputs(x, in_w, out_w, mix_w):
    x = np.ascontiguousarray(x, np.float32)
    in_w = np.asarray(in_w, np.float32)
    out_w = np.asarray(out_w, np.float32)
    mix_w = np.ascontiguousarray(mix_w, np.float32)
    group = []
    for g in range(2):
        ch = _core_channels(g)
        group.append({
            "w_in": np.ascontiguousarray(in_w[ch, :].T).astype(bfloat16),
            "w_out": np.ascontiguousarray(out_w[:, ch].T).astype(bfloat16),
            "pre": mix_w[8 + 4 * g:12 + 4 * g].astype(bfloat16),
            "post": mix_w[4 * g:4 + 4 * g].astype(bfloat16),
        })
    xb = [np.ascontiguousarray(x[b]).astype(bfloat16) for b in range(B)]
    in_maps = []
    for b in range(B):
        for g in range(2):
            m = {"x": xb[b]}
            m.update(group[g])
            in_maps.append(m)
    return in_maps


def _bias_contribution(in_b, out_b, mix_b, mix_w, out_w):
    """Closed-form (E, S) addend from the (linear) bias terms."""
    if not (np.any(in_b) or np.any(out_b) or np.any(mix_b)):
        return None
    s_idx = np.arange(S, dtype=np.float64)
    bias1 = np.zeros((E, S), np.float64)
    for h in range(H):
        cs = slice(h * 64, (h + 1) * 64)
        v = np.asarray(mix_w[h], np.float64)
        if h < H // 2:
            g = (s_idx + 1.0) * v          # cumsum of constant, then *v[s]
        else:
            g = np.cumsum(v)               # cumsum of v[t]
        bias1[cs] = np.asarray(in_b, np.float64)[cs, None] * g[None, :]
        bias1[cs] += np.asarray(mix_b[h], np.float64)[None, :]
    fb = np.asarray(out_w, np.float64) @ bias1
    fb += np.asarray(out_b, np.float64)[:, None]
    return fb.astype(np.float32)


def run_sharded(in_maps, trace=False):
    from concourse.bass_utils import run_bass_kernel_spmd

    return run_bass_kernel_spmd(
        _get_nc(), in_maps, core_ids=list(range(N_CORES)), trace=trace
    )


def gather_output(results, bias_fb=None):
    out = np.empty((B, E, S), np.float32)
    for b in range(B):
        out[b] = results[2 * b]["out"].astype(np.float32)
        out[b] += results[2 * b + 1]["out"].astype(np.float32)
        if bias_fb is not None:
            out[b] += bias_fb
    return out


def kernel(x, in_w, in_b, out_w, out_b, mix_w, mix_b):
    in_maps = shard_inputs(x, in_w, out_w, mix_w)
    res = run_sharded(in_maps, trace=False)
    fb = _bias_contribution(
        np.asarray(in_b), np.asarray(out_b), np.asarray(mix_b),
        np.asarray(mix_w), np.asarray(out_w))
    return gather_output(res.results, fb)
